# revision 12
# baseline (speedup 1.0000x reference)
"""Trainium2 Bass kernel for nn_Attention: GPT-2 style attention block.

Per-core work (data-parallel over batch, 1 of 8 batch elements per core):
  qkv = x @ wa + ba ; split q,k,v into 12 heads of 64
  S_h = q_h @ k_h^T            (no 1/sqrt(D) scaling)
  S masked multiplicatively with tril (masked entries ~= 0, still in softmax!)
  P = softmax(S) ; a_h = P @ v_h
  merged[t, d*12+h] = a_h[t, d] ; y = merged @ wp + bp

v2 design (single-exp softmax):
  - Stats pass computes ONLY the per-row max: S[i,j] matmul + tril mask on
    the diagonal block + negated reduce_max (no exp-accum pass at all).
  - m~ = bf16-rounded row max; the S^T pass fuses -m~ into the QK^T matmul
    via a 65th contraction row (ones row in kT, -m~ row in qT), so the exp
    activation directly emits P_unnorm = e^{S-m~} in [j,i] orientation.
  - Z comes for free from the AV matmul: a ones column is appended to every
    v block (and a count column (7-m) to the v-suffix blocks), so PSUM row 64
    of the AV output accumulates Z_i exactly (wedge entries are fixed to
    e^{-m~} by copy_predicated before AV, suffix counts via the count col).
  - Normalization: reciprocal of the Z row (DVE), full-128 partition
    broadcast (GPSIMD; offset-64 broadcast is broken on HW), and a fused
    multiply during the mergedT PSUM->SBUF copy.
  - Separate PSUM pools for the stats chain and the PT/AV chain so the
    per-head max pipeline (DVE) overlaps the attention pipeline (ACT/PE).
  - mergedT/wp2 in bf16 (values O(1..30); rel-err budget allows it).
"""

import math
import sys

sys.path.insert(0, "/opt/trn_rl_repo")

import numpy as np

import concourse.bass as bass
from concourse import bacc
import concourse.mybir as mybir
import concourse.tile as tile
from concourse import bass_utils
from concourse.masks import make_identity

F32 = mybir.dt.float32
F32R = mybir.dt.float32r
BF16 = mybir.dt.bfloat16
AF = mybir.ActivationFunctionType
ALU = mybir.AluOpType

T = 1024
C = 768
H = 12
D = 64
NT = T // 128        # 8 T-chunks
NCC = C // 128       # 6 C-chunks
# PT storage: per head, block b holds score cols [128*b, 1024), width 1024-128b
PT_W = [T - 128 * b for b in range(NT)]
PT_OFF = [sum(PT_W[:b]) for b in range(NT)]
PT_TOT = sum(PT_W)   # 4608


def _patch_act_tables():
    from concourse import bacc as _bacc_mod
    if getattr(_bacc_mod, "_act_tables_patched", False):
        return
    orig = _bacc_mod.get_activation_tables

    def one_set(arch):
        t = orig(arch)
        keep = "natural_log_exp_and_others"
        if keep in t:
            t = {k: (v if k == keep else set()) for k, v in t.items()}
        return t

    _bacc_mod.get_activation_tables = one_set
    _bacc_mod._act_tables_patched = True


def build_nc():
    _patch_act_tables()
    nc = bacc.Bacc("TRN2", target_bir_lowering=False, debug=False, num_devices=8)

    x = nc.dram_tensor("x", [T, C], F32, kind="ExternalInput").ap()
    wa = nc.dram_tensor("wa", [C, 3 * C], F32, kind="ExternalInput").ap()
    ba = nc.dram_tensor("ba", [3 * C], F32, kind="ExternalInput").ap()
    wp = nc.dram_tensor("wp", [C, C], F32, kind="ExternalInput").ap()
    bp = nc.dram_tensor("bp", [C], F32, kind="ExternalInput").ap()
    y = nc.dram_tensor("y", [T, C], F32, kind="ExternalOutput").ap()

    with tile.TileContext(nc) as tc:
        build_attention(tc, x, wa, ba, wp, bp, y)
    nc.compile()
    return nc


def build_attention(tc, x, wa, ba, wp, bp, y):
    nc = tc.nc

    with (
        tc.tile_pool(name="consts", bufs=1) as consts,
        tc.tile_pool(name="persist", bufs=1) as persist,
        tc.tile_pool(name="rows", bufs=1) as rows,
        tc.tile_pool(name="statsps", bufs=2, space="PSUM") as statsps,
    ):
        # ---------------- constants ----------------
        ident = consts.tile([128, 128], F32, tag="ident")
        make_identity(nc, ident)
        # tril[p, f] = 1 where f <= p (keep), else 0   ([i-part, j-free])
        tril = consts.tile([128, 128], F32, tag="tril")
        nc.gpsimd.memset(tril, 1.0)
        nc.gpsimd.affine_select(
            out=tril, in_=tril, compare_op=ALU.is_ge, fill=0.0,
            base=0, pattern=[[-1, 128]], channel_multiplier=1,
        )
        # wedge[p, f] = 1 where p > f  ([j-part, i-free]: masked region j > i)
        wedge = consts.tile([128, 128], mybir.dt.int8, tag="wedge")
        nc.gpsimd.memset(wedge, 1)
        nc.gpsimd.affine_select(
            out=wedge, in_=wedge, compare_op=ALU.is_gt, fill=0,
            base=0, pattern=[[-1, 128]], channel_multiplier=1,
        )
        onesf = consts.tile([1, T], F32, tag="onesf")
        nc.vector.memset(onesf, 1.0)
        onesr = consts.tile([1, 128], F32R, tag="onesr")
        nc.scalar.copy(onesr, onesf[:, 0:128])
        # bav in [0:768], bp in [768:1536]
        bavbp = consts.tile([1, 2 * C], F32R, tag="bavbp")
        nc.scalar.dma_start(
            out=bavbp[:, 0:C],
            in_=ba[2 * C : 3 * C].rearrange("(a c) -> a c", a=1).bitcast(F32R),
        )
        nc.scalar.dma_start(
            out=bavbp[:, C : 2 * C],
            in_=bp.rearrange("(a c) -> a c", a=1).bitcast(F32R),
        )
        # per-partition bias for q/k projection movers: col m = ba[128m:128(m+1)]
        ba_qk = consts.tile([128, 2 * NCC], F32, tag="ba_qk")
        nc.scalar.dma_start(
            out=ba_qk, in_=ba[0 : 2 * C].rearrange("(m p) -> p m", p=128)
        )

        # ---------------- persistent activations ----------------
        # One head per 128-partition chunk: rows 0:64 = q^T (k^T); row 64 of
        # kT = ones, row 64 of qT = -m~ (written per head after stats).
        qT = persist.tile([128, H, T], F32R, tag="qT")
        kT = persist.tile([128, H, T], F32R, tag="kT")
        # v blocks + ones col (col 64) for the Z accumulation
        vz = persist.tile([128, NT, H, D + 1], BF16, tag="vz")
        # negated row maxes, per pair: col half*8+r
        maxs = persist.tile([128, 6, 16], F32, tag="maxs")

        nc.vector.memset(vz[:, :, :, D : D + 1], 1.0)
        for h in range(H):
            nc.sync.dma_start(out=kT[64:65, h, :], in_=onesf.bitcast(F32R))

        # ---------------- stats (negated masked row max) --------------------
        def emit_stats(pair):
            for half in range(2):
                h = 2 * pair + half
                for r in range(NT):
                    jcols = 128 * (r + 1)
                    sps = statsps.tile([128, 1024], F32, tag="st_s")
                    pieces = [(0, min(jcols, 512))]
                    if jcols > 512:
                        pieces.append((512, jcols - 512))
                    for (p0, pw) in pieces:
                        nc.tensor.matmul(
                            sps[:, p0 : p0 + pw],
                            qT[0:64, h, 128 * r : 128 * r + 128],
                            kT[0:64, h, p0 : p0 + pw],
                            start=True,
                            stop=True,
                        )
                    # mask the diag block, then negated row max; masked
                    # entries become 0 so m~ = max(true_max, 0) (valid shift)
                    nc.vector.tensor_mul(
                        sps[:, 128 * r : 128 * r + 128],
                        sps[:, 128 * r : 128 * r + 128],
                        tril,
                    )
                    nc.vector.reduce_max(
                        maxs[:, pair, 8 * half + r : 8 * half + r + 1],
                        sps[:, 0:jcols],
                        axis=mybir.AxisListType.X,
                        negate=True,
                    )

        # ---------------- phase 1: loads + projections ----------------------
        with (
            tc.tile_pool(name="xload", bufs=1) as xload,
            tc.tile_pool(name="projps", bufs=2, space="PSUM") as projps,
        ):
            xT = xload.tile([128, NCC, T], F32R, tag="xT")
            # wa cols [0:1536] (q,k) first; the same slot is later reused
            # for cols [1536:2304] (v) once all q/k units have run.
            wa_qk = xload.tile([128, NCC, 2 * C], F32R, tag="wa_sl")

            def emit_xt(trange):
                for t in trange:
                    xc = xload.tile([128, C], F32, tag="xchunk", bufs=2)
                    nc.sync.dma_start(out=xc, in_=x[128 * t : 128 * t + 128, :])
                    for g in range(2):
                        ps = projps.tile([128, 1024], F32, tag="ps_proj")
                        for q in range(3):
                            cc = 3 * g + q
                            nc.tensor.transpose(
                                ps[:, 128 * q : 128 * q + 128],
                                xc[:, 128 * cc : 128 * cc + 128], ident,
                            )
                        nc.scalar.activation(
                            xT[:, 3 * g : 3 * g + 3, 128 * t : 128 * t + 128],
                            ps[:, 0:384].rearrange("p (c f) -> p c f", c=3),
                            AF.Identity,
                        )

            emit_xt(range(2))
            for cc in range(2):
                nc.scalar.dma_start(
                    out=wa_qk[:, cc, :],
                    in_=wa[128 * cc : 128 * cc + 128, 0 : 2 * C].bitcast(F32R),
                )
            emit_xt(range(2, NT))
            for cc in range(2, NCC):
                nc.scalar.dma_start(
                    out=wa_qk[:, cc, :],
                    in_=wa[128 * cc : 128 * cc + 128, 0 : 2 * C].bitcast(F32R),
                )

            def emit_unit(m):
                # feature block [128m, 128m+128): q heads (2m, 2m+1) for m<6,
                # k heads (2(m-6), 2(m-6)+1) for m>=6
                ps = projps.tile([128, 1024], F32, tag="ps_proj")
                for nhalf in range(2):
                    for cc in range(NCC):
                        nc.tensor.matmul(
                            ps[:, 512 * nhalf : 512 * nhalf + 512],
                            wa_qk[:, cc, 128 * m : 128 * m + 128],
                            xT[:, cc, 512 * nhalf : 512 * nhalf + 512],
                            start=(cc == 0),
                            stop=(cc == NCC - 1),
                        )
                dest = qT if m < 6 else kT
                h0 = 2 * (m % 6)
                for par in range(2):
                    nc.scalar.activation(
                        dest[0:64, h0 + par, :],
                        ps[64 * par : 64 * par + 64, :],
                        AF.Identity,
                        bias=ba_qk[64 * par : 64 * par + 64, m : m + 1],
                    )

            for p in range(6):
                emit_unit(p)
                emit_unit(6 + p)
                if p >= 1:
                    emit_stats(p - 1)

            # v weights overwrite the q/k slot (tile framework adds the
            # WAR dependency on all prior readers)
            wa_v = xload.tile([128, NCC, 2 * C], F32R, tag="wa_sl")
            for cc in range(NCC):
                nc.scalar.dma_start(
                    out=wa_v[:, cc, 0:C],
                    in_=wa[128 * cc : 128 * cc + 128, 2 * C : 3 * C].bitcast(F32R),
                )

            for t in range(NT):
                ps = projps.tile([128, 1024], F32, tag="ps_proj")
                for (n, o0) in ((0, 0), (1, 512)):
                    for cc in range(NCC):
                        nc.tensor.matmul(
                            ps[:, o0 : o0 + 384],
                            xT[:, cc, 128 * t : 128 * t + 128],
                            wa_v[:, cc, 384 * n : 384 * n + 384],
                            start=(cc == 0),
                            stop=False,
                        )
                    nc.tensor.matmul(
                        ps[:, o0 : o0 + 384],
                        onesr,
                        bavbp[:, 384 * n : 384 * n + 384],
                        start=False,
                        stop=True,
                    )
                    nc.scalar.activation(
                        vz[:, t, 6 * n : 6 * n + 6, 0:D],
                        ps[:, o0 : o0 + 384].rearrange("p (h d) -> p h d", d=D),
                        AF.Identity,
                    )
            emit_stats(5)

        # -------- attn phase ------------------------------------------------
        with tc.tile_pool(name="late", bufs=1) as late:
            mergedT = late.tile([128, NCC, T], BF16, tag="mergedT")
            wp2 = late.tile([128, NCC, C], BF16, tag="wp2")
            with (
                tc.tile_pool(name="attnsb", bufs=1) as attnsb,
                tc.tile_pool(name="ptpool", bufs=1) as ptpool,
                tc.tile_pool(name="bexpool", bufs=2) as bexpool,
                tc.tile_pool(name="zpool", bufs=2) as zpool,
                tc.tile_pool(name="ptav", bufs=2, space="PSUM") as ptav,
            ):
                # v_suf[m] = sum of v blocks b > m; count col 64 = (7-m)
                vsuf = attnsb.tile([128, NT - 1, H, D + 1], BF16, tag="vsuf")
                nc.vector.tensor_copy(vsuf[:, 6, :, 0:D], vz[:, 7, :, 0:D])
                for m in range(5, -1, -1):
                    nc.vector.tensor_add(
                        vsuf[:, m, :, 0:D], vsuf[:, m + 1, :, 0:D],
                        vz[:, m + 1, :, 0:D],
                    )
                for m in range(NT - 1):
                    nc.vector.memset(vsuf[:, m, :, D : D + 1], float(NT - 1 - m))

                # wp2 load (row-permuted: merged col c2=h*64+d <-> wp row d*12+h)
                wp_r = wp.rearrange("(d h) c -> d h c", h=H)  # [64, 12, 768]
                for k in range(NCC):
                    wst = attnsb.tile([128, C], F32, tag="wpst", bufs=2)
                    for par in range(2):
                        nc.sync.dma_start(
                            out=wst[64 * par : 64 * par + 64, :],
                            in_=wp_r[:, 2 * k + par, :],
                        )
                    nc.scalar.copy(wp2[:, k, :], wst)

                def emit_rows(pair):
                    # -m~ (bf16-rounded) f32r row + e^{-m~} rows
                    pst = statsps.tile([128, 1024], F32, tag="st_s")
                    nc.tensor.transpose(pst[0:16, 0:128], maxs[:, pair, :], ident)
                    rowb = rows.tile([16, 128], BF16, tag="rowb")
                    nc.scalar.copy(rowb, pst[0:16, 0:128])
                    rowr = rows.tile([16, 128], F32R, tag="rowr")
                    nc.scalar.copy(rowr, rowb)
                    expm = rows.tile([16, 128], BF16, tag="expm")
                    nc.scalar.activation(expm, rowb, AF.Exp)
                    erowp = rows.tile([1, 2 * T], BF16, tag="erowp")
                    for half in range(2):
                        h = 2 * pair + half
                        nc.sync.dma_start(
                            out=qT[64:65, h, :].rearrange("a (p f) -> a p f", p=8),
                            in_=rowr[8 * half : 8 * half + 8, :],
                        )
                    nc.sync.dma_start(
                        out=erowp.rearrange("a (p f) -> a p f", p=16), in_=expm
                    )
                    return erowp

                def emit_pt(pair, half, erowp):
                    h = 2 * pair + half
                    bexp = bexpool.tile([128, T], BF16, tag=f"bexp{half}")
                    nc.gpsimd.partition_broadcast(
                        bexp, erowp[:, T * half : T * half + T], channels=128
                    )
                    pt = ptpool.tile([128, PT_TOT], BF16, tag=f"pt{half}")
                    for b in range(NT):
                        w = T - 128 * b
                        # pieces split so each dest region stays in-bank
                        if b == 0:
                            pieces = [(0, 512), (512, 512)]
                        elif b < 4:
                            pieces = [
                                (128 * b, 512 - 128 * b),
                                (512, 128 * b),
                                (512 + 128 * b, 512 - 128 * b),
                            ]
                        else:
                            pieces = [(128 * b, w)]
                        sps = ptav.tile([128, 1024], F32, tag="pa")
                        for (g0, gw) in pieces:
                            # K=65: row 64 of kT is ones, row 64 of qT is -m~
                            nc.tensor.matmul(
                                sps[:, g0 - 128 * b : g0 - 128 * b + gw],
                                kT[0:65, h, 128 * b : 128 * b + 128],
                                qT[0:65, h, g0 : g0 + gw],
                                start=True,
                                stop=True,
                            )
                        nc.scalar.activation(
                            pt[:, PT_OFF[b] : PT_OFF[b] + w], sps[:, 0:w],
                            AF.Exp,
                        )
                        # wedge of diag block -> e^{-m~}
                        nc.vector.copy_predicated(
                            pt[:, PT_OFF[b] : PT_OFF[b] + 128],
                            wedge,
                            bexp[:, 128 * b : 128 * b + 128],
                        )
                    return pt, bexp

                def emit_av(pair, half, c, pt, bexp):
                    h = 2 * pair + half
                    c0, c1 = 512 * c, 512 * c + 512
                    zrec = zpool.tile([1, 512], F32, tag="zrec")
                    zb = zpool.tile([128, 512], F32, tag="zb")
                    ps = ptav.tile([128, 1024], F32, tag="pa")
                    mms = []
                    for b in range(NT):
                        lo_blk = 128 * b
                        if lo_blk >= c1:
                            continue
                        g0 = max(lo_blk, c0)
                        lo = PT_OFF[b] + g0 - lo_blk
                        mms.append(
                            (vz[:, b, h, :], pt[:, lo : lo + (c1 - g0)], g0 - c0)
                        )
                    for m in range(4 * c, min(4 * c + 4, 7)):
                        mms.append(
                            (vsuf[:, m, h, :],
                             bexp[:, 128 * m : 128 * m + 128],
                             128 * m - c0)
                        )
                    for idx, (lhsT, rhs, off) in enumerate(mms):
                        nw = rhs.shape[-1]
                        nc.tensor.matmul(
                            ps[0:65, off : off + nw],
                            lhsT, rhs,
                            start=(idx == 0),
                            stop=(idx == len(mms) - 1),
                        )
                    # normalization: 1/Z, full-128 broadcast, fused mul
                    nc.vector.reciprocal(zrec, ps[64:65, 0:512])
                    nc.gpsimd.partition_broadcast(zb, zrec, channels=128)
                    nc.vector.tensor_mul(
                        mergedT[64 * half : 64 * half + 64, pair, c0:c1],
                        ps[0:64, 0:512],
                        zb[64 * half : 64 * half + 64, :],
                    )

                def emit_cproj(trange, ysbp):
                    for t in trange:
                        ps = statsps.tile([128, 1024], F32, tag="st_s")
                        for (n0, nw) in ((0, 512), (512, 256)):
                            for k in range(NCC):
                                nc.tensor.matmul(
                                    ps[:, n0 : n0 + nw],
                                    mergedT[:, k, 128 * t : 128 * t + 128],
                                    wp2[:, k, n0 : n0 + nw],
                                    start=(k == 0),
                                    stop=False,
                                )
                            nc.tensor.matmul(
                                ps[:, n0 : n0 + nw],
                                onesr,
                                bavbp[:, C + n0 : C + n0 + nw],
                                start=False,
                                stop=True,
                            )
                        yt = ysbp.tile([128, C], F32, tag="y_stage")
                        nc.scalar.activation(yt, ps[:, 0:C], AF.Identity)
                        nc.sync.dma_start(
                            out=y[128 * t : 128 * t + 128, :], in_=yt
                        )

                with tc.tile_pool(name="ysb", bufs=2) as ysbp:
                    einfo = {0: emit_rows(0)}
                    for p in range(6):
                        if p + 1 < 6:
                            einfo[p + 1] = emit_rows(p + 1)
                        erowp = einfo.pop(p)
                        avs = []
                        for half in range(2):
                            pt, bexp = emit_pt(p, half, erowp)
                            for c in range(2):
                                if p == 5 and half == 1:
                                    avs.append((half, c, pt, bexp))
                                else:
                                    emit_av(p, half, c, pt, bexp)
                        if p == 5:
                            # interleave the last pair's AV with c_proj
                            half, c, pt, bexp = avs[0]
                            emit_av(p, half, c, pt, bexp)
                            emit_cproj(range(0, 4), ysbp)
                            half, c, pt, bexp = avs[1]
                            emit_av(p, half, c, pt, bexp)
                            emit_cproj(range(4, NT), ysbp)


_NC_CACHE = None


def get_nc():
    global _NC_CACHE
    if _NC_CACHE is None:
        _NC_CACHE = build_nc()
    return _NC_CACHE


def kernel(x, wa, ba, wp, bp, **kw):
    x = np.asarray(x, dtype=np.float32)
    in_maps = [
        {
            "x": np.ascontiguousarray(x[b]),
            "wa": np.asarray(wa, dtype=np.float32),
            "ba": np.asarray(ba, dtype=np.float32),
            "wp": np.asarray(wp, dtype=np.float32),
            "bp": np.asarray(bp, dtype=np.float32),
        }
        for b in range(8)
    ]
    res = bass_utils.run_bass_kernel_spmd(get_nc(), in_maps, core_ids=list(range(8)))
    return np.stack([r["y"] for r in res.results], axis=0)


if __name__ == "__main__":
    nc = build_nc()
    print("build OK")
